# revision 13
# baseline (speedup 1.0000x reference)
"""Trainium2 Bass kernel for the HNN leapfrog dynamical-inference layer.

Reference: 3 leapfrog steps over phase space zp=[q,p], H(zp)=sum(MLP(zp)),
MLP = tanh(zp@W1+b1) -> tanh(@W2+b2) -> @W3+b3; 3 gradient evals per step.

Algebraic restructuring (validated ~5e-8 rel err vs reference in fp32):
  - p0 = 0 and q,p only enter through a = zp@W1, so track the 256-dim
    pre-activation state T = q@W1q + p@W1p instead of q,p.
  - kick:  p -= c*gq  =>  T += u1s @ (W1q^T W1p)  (Mqp precomputed)
  - drift: q += dt*gp =>  T += u1s @ (W1p^T W1q)  (Mpq precomputed)
    u1s = scale*(1-h1^2)*((1-h2^2)*w3 @ W2^T), integration scale folded.
  - q_final = z + (sum over drift evals of u1s) @ W1p^T; last kick dead.
  - (1-h2^2)*w3 @ W2^T = C + h2^2 @ W2wneg with C = W2@w3,
    W2wneg[j,i] = -w3[j]*W2[i,j]; scale folded into per-eval-kind copies
    of W2wneg/C on the host; C enters the PSUM accumulation as a rank-1
    matmul (C x ones), so the u2/v stage costs zero elementwise ops.
  - u1s = (h1^2-1) * v as one scalar_tensor_tensor reading the backward
    matmul PSUM directly.
  - T lives PERMANENTLY in PSUM: the update matmuls accumulate into the
    same PSUM banks (start=False) on top of the init z@W1q group, and
    tanh1 reads T straight from PSUM. No state add/copy ops exist at all.
The batch is processed in 4 passes of 512 columns per core; each pass
owns 2 PSUM banks for its T state, two passes in flight at a time
(4 banks T + 4 rotating matmul banks = full PSUM).
All matmuls bf16 (full PE rate); T accumulates fp32 in PSUM; the final
q = z + s@W1pT add is fp32 with unrounded z. Gradient-path bf16 enters
the output scaled by ~0.006 -> ~2.5e-5 end-to-end relative error.

Layout: activations transposed (features on partitions, batch on free);
weights host-pretransposed; no on-device transposes.
Sharding: pure data parallel, batch 16384 -> 8 cores x 2048 rows.
"""

import numpy as np
import ml_dtypes

import concourse.mybir as mybir
import concourse.tile as tile
from concourse import bacc
from concourse.bass_utils import run_bass_kernel_spmd

AF = mybir.ActivationFunctionType
ALU = mybir.AluOpType
FP32 = mybir.dt.float32
BF16 = mybir.dt.bfloat16
BF = ml_dtypes.bfloat16

N_CORES = 8
B, DIM, HID = 16384, 512, 256
DT = 0.1
BL = B // N_CORES            # batch rows per core (2048)
NP = 4                       # batch passes per core
PW = BL // NP                # pass width (512 cols)
KD = DIM // 128              # k-tiles over q-features (4)
KH = HID // 128              # k-tiles over hidden (2)
MQ = DIM // 128              # m-tiles over final q-features (4)

EVALS = ["k", "d", "k", "k", "d", "k", "k", "d"]

# engine placement for the squares (per m-tile): h1^2 and h2^2
SQ1_ENGINES = ["act", "dve"]
SQ2_ENGINES = ["dve", "dve"]


def build_nc():
    nc = bacc.Bacc("TRN2", target_bir_lowering=False, debug=False)

    zT_d = nc.dram_tensor("zT", [DIM, BL], FP32, kind="ExternalInput")
    zTb_d = nc.dram_tensor("zTb", [DIM, BL], BF16, kind="ExternalInput")
    w1q_d = nc.dram_tensor("w1q", [128, KD, HID], BF16, kind="ExternalInput")
    w2_d = nc.dram_tensor("w2", [128, KH, HID], BF16, kind="ExternalInput")
    wk_d = nc.dram_tensor("wk", [128, KH, HID], BF16, kind="ExternalInput")
    wd_d = nc.dram_tensor("wd", [128, KH, HID], BF16, kind="ExternalInput")
    mqp_d = nc.dram_tensor("mqp", [128, KH, HID], BF16, kind="ExternalInput")
    mpq_d = nc.dram_tensor("mpq", [128, KH, HID], BF16, kind="ExternalInput")
    w1pt_d = nc.dram_tensor("w1pt", [128, KH, DIM], BF16, kind="ExternalInput")
    b1_d = nc.dram_tensor("b1", [128, KH], FP32, kind="ExternalInput")
    b2_d = nc.dram_tensor("b2", [128, KH], FP32, kind="ExternalInput")
    ckr_d = nc.dram_tensor("ckr", [1, HID], BF16, kind="ExternalInput")
    cdr_d = nc.dram_tensor("cdr", [1, HID], BF16, kind="ExternalInput")
    qT_d = nc.dram_tensor("qT", [DIM, BL], FP32, kind="ExternalOutput")

    with tile.TileContext(nc) as tc:
        with (
            tc.tile_pool(name="const", bufs=1) as cp,
            tc.tile_pool(name="state", bufs=1) as sp,
            tc.tile_pool(name="work", bufs=2) as wp,
            tc.tile_pool(name="qo", bufs=4) as qp,
            tc.tile_pool(name="ps", bufs=4, space="PSUM") as pp,
            tc.tile_pool(name="pst", bufs=4, space="PSUM") as pt,
        ):
            # ---- weights / biases on the gpsimd DMA queue
            w1q = cp.tile([128, KD, HID], BF16, tag="w1q", name="w1q")
            nc.gpsimd.dma_start(w1q[:], w1q_d.ap()[:])
            w2 = cp.tile([128, KH, HID], BF16, tag="w2", name="w2")
            nc.gpsimd.dma_start(w2[:], w2_d.ap()[:])
            wk = cp.tile([128, KH, HID], BF16, tag="wk", name="wk")
            nc.gpsimd.dma_start(wk[:], wk_d.ap()[:])
            wd = cp.tile([128, KH, HID], BF16, tag="wd", name="wd")
            nc.gpsimd.dma_start(wd[:], wd_d.ap()[:])
            mqp = cp.tile([128, KH, HID], BF16, tag="mqp", name="mqp")
            nc.gpsimd.dma_start(mqp[:], mqp_d.ap()[:])
            mpq = cp.tile([128, KH, HID], BF16, tag="mpq", name="mpq")
            nc.gpsimd.dma_start(mpq[:], mpq_d.ap()[:])
            b1 = cp.tile([128, KH], FP32, tag="b1", name="b1")
            nc.gpsimd.dma_start(b1[:], b1_d.ap()[:])
            b2 = cp.tile([128, KH], FP32, tag="b2", name="b2")
            nc.gpsimd.dma_start(b2[:], b2_d.ap()[:])
            ckr = cp.tile([1, HID], BF16, tag="ckr", name="ckr")
            nc.gpsimd.dma_start(ckr[:], ckr_d.ap()[:])
            cdr = cp.tile([1, HID], BF16, tag="cdr", name="cdr")
            nc.gpsimd.dma_start(cdr[:], cdr_d.ap()[:])
            w1pt = cp.tile([128, KH, DIM], BF16, tag="w1pt", name="w1pt")
            nc.gpsimd.dma_start(w1pt[:], w1pt_d.ap()[:])
            ones = cp.tile([1, 512], BF16, tag="ones", name="ones")
            nc.vector.memset(ones[:], 1.0)

            # bf16 z, split per pass so pass 0 can start early
            zTb = [
                sp.tile([128, BL], BF16, tag=f"zTb{k}", name=f"zTb{k}")
                for k in range(KD)
            ]
            for p in range(NP):
                for k in range(KD):
                    nc.sync.dma_start(
                        zTb[k][:, p * PW : (p + 1) * PW],
                        zTb_d.ap()[k * 128 : (k + 1) * 128, p * PW : (p + 1) * PW],
                    )

            # fp32 z, needed only by the finals (low-priority queue order)
            zT = [
                sp.tile([128, BL], FP32, tag=f"zT{k}", name=f"zT{k}")
                for k in range(KD)
            ]
            for k in range(KD):
                nc.gpsimd.dma_start(zT[k][:], zT_d.ap()[k * 128 : (k + 1) * 128, :])

            def square(eng, out, src):
                if eng == "act":
                    nc.scalar.activation(out[:], src[:], AF.Square)
                else:
                    nc.vector.tensor_mul(out[:], src[:], src[:])

            # ---- per-pass pipeline; T state lives in PSUM the whole pass
            for p in range(NP):
                psl = slice(p * PW, (p + 1) * PW)

                # T init: T = z @ W1q  (p0 = 0)
                Tp = [
                    pt.tile([128, PW], FP32, tag="Tps", name=f"Tps{p}_{m}")
                    for m in range(KH)
                ]
                for m in range(KH):
                    for k in range(KD):
                        nc.tensor.matmul(
                            Tp[m][:],
                            w1q[:, k, m * 128 : (m + 1) * 128],
                            zTb[k][:, psl],
                            start=(k == 0),
                            stop=(k == KD - 1),
                            skip_group_check=True,
                        )

                sb = [
                    sp.tile([128, PW], BF16, tag=f"s{p}_{m}", name=f"s{p}_{m}")
                    for m in range(KH)
                ]

                for ei, kind in enumerate(EVALS):
                    wv = wk if kind == "k" else wd
                    cr = ckr if kind == "k" else cdr
                    updw = mqp if kind == "k" else mpq
                    first_drift = kind == "d" and ei == 1
                    is_last = ei == len(EVALS) - 1

                    h1 = [
                        wp.tile([128, PW], BF16, tag=f"h1_{p%2}_{m}",
                                name=f"h1_{p}_{m}")
                        for m in range(KH)
                    ]
                    sq1 = [
                        wp.tile([128, PW], BF16, tag=f"sq1_{p%2}_{m}",
                                name=f"sq1_{p}_{m}")
                        for m in range(KH)
                    ]
                    h2 = [
                        wp.tile([128, PW], BF16, tag=f"h2_{p%2}_{m}",
                                name=f"h2_{p}_{m}")
                        for m in range(KH)
                    ]
                    sq2 = [
                        wp.tile([128, PW], BF16, tag=f"sq2_{p%2}_{m}",
                                name=f"sq2_{p}_{m}")
                        for m in range(KH)
                    ]
                    u1 = [
                        wp.tile([128, PW], BF16, tag=f"u1_{p%2}_{m}",
                                name=f"u1_{p}_{m}")
                        for m in range(KH)
                    ]

                    # h1 = tanh(T + b1) straight from the T PSUM banks
                    for m in range(KH):
                        nc.scalar.activation(
                            h1[m][:], Tp[m][:], AF.Tanh, bias=b1[:, m : m + 1]
                        )
                    for m in range(KH):
                        square(SQ1_ENGINES[m], sq1[m], h1[m])

                    # h2 = tanh(h1 @ W2 + b2); sq2 = h2^2
                    for m in range(KH):
                        ps = pp.tile([128, PW], FP32, tag="mm", name="mm")
                        for k in range(KH):
                            nc.tensor.matmul(
                                ps[:],
                                w2[:, k, m * 128 : (m + 1) * 128],
                                h1[k][:],
                                start=(k == 0),
                                stop=(k == KH - 1),
                            )
                        nc.scalar.activation(
                            h2[m][:], ps[:], AF.Tanh, bias=b2[:, m : m + 1]
                        )
                    for m in range(KH):
                        square(SQ2_ENGINES[m], sq2[m], h2[m])

                    # v = scale*(C + sq2 @ W2wneg) via pre-scaled wv + rank-1 C
                    # u1 = (sq1 - 1) * v straight off PSUM
                    for m in range(KH):
                        ps = pp.tile([128, PW], FP32, tag="mm", name="mm")
                        for k in range(KH):
                            nc.tensor.matmul(
                                ps[:],
                                wv[:, k, m * 128 : (m + 1) * 128],
                                sq2[k][:],
                                start=(k == 0),
                                stop=False,
                            )
                        nc.tensor.matmul(
                            ps[:],
                            cr[:, m * 128 : (m + 1) * 128],
                            ones[:],
                            start=False,
                            stop=True,
                        )
                        nc.vector.scalar_tensor_tensor(
                            u1[m][:], sq1[m][:], 1.0, ps[:],
                            ALU.subtract, ALU.mult,
                        )

                    # s accumulation on drift evals (bf16)
                    if kind == "d":
                        for m in range(KH):
                            if first_drift:
                                nc.vector.tensor_copy(sb[m][:], u1[m][:])
                            else:
                                nc.vector.tensor_add(sb[m][:], sb[m][:], u1[m][:])

                    # T += u1 @ updw, accumulated in place in the T PSUM banks
                    if not is_last:
                        for m in range(KH):
                            for k in range(KH):
                                nc.tensor.matmul(
                                    Tp[m][:],
                                    updw[:, k, m * 128 : (m + 1) * 128],
                                    u1[k][:],
                                    start=False,
                                    stop=(k == KH - 1),
                                    skip_group_check=True,
                                )

                # final for this pass: q = z + s @ W1p^T
                for mq in range(MQ):
                    ps = pp.tile([128, PW], FP32, tag="mm", name="mm")
                    for k in range(KH):
                        nc.tensor.matmul(
                            ps[:],
                            w1pt[:, k, mq * 128 : (mq + 1) * 128],
                            sb[k][:],
                            start=(k == 0),
                            stop=(k == KH - 1),
                        )
                    qo = qp.tile([128, PW], FP32, tag="qo", name="qo")
                    nc.vector.tensor_add(qo[:], zT[mq][:, psl], ps[:])
                    nc.sync.dma_start(
                        qT_d.ap()[mq * 128 : (mq + 1) * 128, psl], qo[:]
                    )

    nc.compile()
    return nc


_CACHE = {}


def _get_nc():
    if "nc" not in _CACHE:
        _CACHE["nc"] = build_nc()
    return _CACHE["nc"]


def _tile_k(a, ktiles):
    """[K, M] -> [128, ktiles, M] with K = ktiles*128 on partitions."""
    k, m = a.shape
    assert k == ktiles * 128
    return np.ascontiguousarray(a.reshape(ktiles, 128, m).transpose(1, 0, 2))


def _bias_tiles(v):
    """[256] -> [128, 2]: column m holds features m*128..(m+1)*128."""
    return np.ascontiguousarray(v.reshape(KH, 128).T)


def _prep_shared(W1, b1, W2, b2, W3, b3):
    W1 = np.asarray(W1, dtype=np.float32)
    W2 = np.asarray(W2, dtype=np.float32)
    w3 = np.asarray(W3, dtype=np.float32)[:, 0]
    b1 = np.asarray(b1, dtype=np.float32)
    b2 = np.asarray(b2, dtype=np.float32)
    W1q, W1p = W1[:DIM], W1[DIM:]
    W2wneg = -(w3[:, None] * W2.T)
    C = W2 @ w3
    Mqp = W1q.T @ W1p
    Mpq = W1p.T @ W1q
    # kick: u1 = (h1^2-1) * (dt/2)*(C + sq2@W2wneg)   (scale -dt/2 folded)
    # drift: u1 = (h1^2-1) * (-dt)*(C + sq2@W2wneg)   (scale +dt folded)
    return {
        "w1q": _tile_k(W1q, KD).astype(BF),
        "w2": _tile_k(W2, KH).astype(BF),
        "wk": _tile_k((DT / 2) * W2wneg, KH).astype(BF),
        "wd": _tile_k((-DT) * W2wneg, KH).astype(BF),
        "mqp": _tile_k(Mqp, KH).astype(BF),
        "mpq": _tile_k(Mpq, KH).astype(BF),
        "w1pt": _tile_k(np.ascontiguousarray(W1p.T), KH).astype(BF),
        "b1": _bias_tiles(b1),
        "b2": _bias_tiles(b2),
        "ckr": ((DT / 2) * C).reshape(1, HID).astype(BF),
        "cdr": ((-DT) * C).reshape(1, HID).astype(BF),
    }


def run_kernel(z, W1, b1, W2, b2, W3, b3, trace=False, trace_cores=None):
    nc = _get_nc()
    shared = _prep_shared(W1, b1, W2, b2, W3, b3)
    z = np.asarray(z, dtype=np.float32)
    in_maps = []
    for i in range(N_CORES):
        zt = np.ascontiguousarray(z[i * BL : (i + 1) * BL].T)
        in_maps.append({**shared, "zT": zt, "zTb": zt.astype(BF)})
    res = run_bass_kernel_spmd(
        nc,
        in_maps,
        core_ids=list(range(N_CORES)),
        trace=trace,
        trace_cores=trace_cores,
    )
    out = np.concatenate([res.results[i]["qT"].T for i in range(N_CORES)], axis=0)
    return np.ascontiguousarray(out), res


def kernel(z, W1, b1, W2, b2, W3, b3):
    out, _ = run_kernel(z, W1, b1, W2, b2, W3, b3)
    return out


# revision 15
# speedup vs baseline: 1.2723x; 1.2723x over previous
"""Trainium2 Bass kernel for the HNN leapfrog dynamical-inference layer.

Reference computation: 3 leapfrog steps over phase space zp=[q,p] with
H(zp) = sum(MLP(zp)), MLP = tanh(zp@W1+b1) -> tanh(@W2+b2) -> @W3+b3.
Each step does 3 gradient evals of H (kick/drift/kick).

Key algebraic restructuring (validated to ~5e-8 rel err vs reference):
  - p starts at 0 and q/p only enter the network through a = zp@W1, so we
    track the 256-dim state T = q@W1q + p@W1p instead of q,p themselves.
  - kick:  p -= c*gq  =>  T += u1s @ (W1q^T W1p)   (Mqp, precomputed)
  - drift: q += dt*gp =>  T += u1s @ (W1p^T W1q)   (Mpq, precomputed)
    where u1s = scale*(1-h1^2)*((1-h2^2)*w3 @ W2^T) is the layer-1 adjoint
    with the integration constant folded in.
  - Output q_final = z + (sum of drift u1s) @ W1p^T  -- only the s
    accumulator is needed; the last kick (eval 9) is dead and skipped.
  - (1-h2^2)*w3 @ W2^T = C + h2^2 @ W2wneg with C = W2@w3,
    W2wneg[j,i] = -w3[j]*W2[i,j]  (both precomputed on host), so no
    elementwise op for the u2 stage is needed at all.
This cuts matmul FLOPs ~3.3x vs the naive chain. All matmuls run in bf16
(full PE rate); state T, s and the final q = z + ... add stay fp32, so the
bf16 rounding only perturbs the gradient path, which enters the output
scaled by ~dt^.. (|q-z| ~ 0.006*|z|): measured end-to-end error ~2.5e-5.

Layout: activations transposed -- features on partitions, batch on the
free axis -- so every matmul uses host-pretransposed weights as the
stationary operand and no on-device transposes are needed anywhere.
Sharding: pure data parallel, batch 16384 -> 8 cores x 2048.
"""

import numpy as np
import ml_dtypes

import concourse.mybir as mybir
import concourse.tile as tile
from concourse import bacc
from concourse.bass_utils import run_bass_kernel_spmd

AF = mybir.ActivationFunctionType
ALU = mybir.AluOpType
FP32 = mybir.dt.float32
BF16 = mybir.dt.bfloat16
BF = ml_dtypes.bfloat16

N_CORES = 8
B, DIM, HID = 16384, 512, 256
DT = 0.1
BL = B // N_CORES            # batch rows per core (2048)
NCHUNK = 4                   # batch chunks per core
CH = BL // NCHUNK            # batch cols per chunk (512)
KD = DIM // 128              # k-tiles over q-features (4)
KH = HID // 128              # k-tiles over hidden (2)
MQ = DIM // 128              # m-tiles over output q-features (4)

# eval sequence after dropping the dead final kick: k=kick, d=drift
EVALS = ["k", "d", "k", "k", "d", "k", "k", "d"]


def build_nc():
    nc = bacc.Bacc("TRN2", target_bir_lowering=False, debug=False)

    zT_d = nc.dram_tensor("zT", [DIM, BL], FP32, kind="ExternalInput")
    zTb_d = nc.dram_tensor("zTb", [DIM, BL], BF16, kind="ExternalInput")
    w1q_d = nc.dram_tensor("w1q", [128, KD, HID], BF16, kind="ExternalInput")
    w2_d = nc.dram_tensor("w2", [128, KH, HID], BF16, kind="ExternalInput")
    w2wn_d = nc.dram_tensor("w2wn", [128, KH, HID], BF16, kind="ExternalInput")
    mqp_d = nc.dram_tensor("mqp", [128, KH, HID], BF16, kind="ExternalInput")
    mpq_d = nc.dram_tensor("mpq", [128, KH, HID], BF16, kind="ExternalInput")
    w1pt_d = nc.dram_tensor("w1pt", [128, KH, DIM], BF16, kind="ExternalInput")
    b1_d = nc.dram_tensor("b1", [128, KH], FP32, kind="ExternalInput")
    b2_d = nc.dram_tensor("b2", [128, KH], FP32, kind="ExternalInput")
    ck_d = nc.dram_tensor("ck", [128, KH], FP32, kind="ExternalInput")
    cd_d = nc.dram_tensor("cd", [128, KH], FP32, kind="ExternalInput")
    qT_d = nc.dram_tensor("qT", [DIM, BL], FP32, kind="ExternalOutput")

    with tile.TileContext(nc) as tc:
        with (
            tc.tile_pool(name="const", bufs=1) as cp,
            tc.tile_pool(name="state", bufs=1) as sp,
            tc.tile_pool(name="work", bufs=1) as wp,
            tc.tile_pool(name="qo", bufs=4) as qp,
            tc.tile_pool(name="ps", bufs=6, space="PSUM") as pp,
            tc.tile_pool(name="psf", bufs=2, space="PSUM") as pf,
        ):
            # ---- weights / biases (tiny, land first)
            w1q = cp.tile([128, KD, HID], BF16, tag="w1q", name="w1q")
            nc.gpsimd.dma_start(w1q[:], w1q_d.ap()[:])
            w2 = cp.tile([128, KH, HID], BF16, tag="w2", name="w2")
            nc.gpsimd.dma_start(w2[:], w2_d.ap()[:])
            w2wn = cp.tile([128, KH, HID], BF16, tag="w2wn", name="w2wn")
            nc.gpsimd.dma_start(w2wn[:], w2wn_d.ap()[:])
            mqp = cp.tile([128, KH, HID], BF16, tag="mqp", name="mqp")
            nc.gpsimd.dma_start(mqp[:], mqp_d.ap()[:])
            mpq = cp.tile([128, KH, HID], BF16, tag="mpq", name="mpq")
            nc.gpsimd.dma_start(mpq[:], mpq_d.ap()[:])
            w1pt = cp.tile([128, KH, DIM], BF16, tag="w1pt", name="w1pt")
            nc.gpsimd.dma_start(w1pt[:], w1pt_d.ap()[:])
            b1 = cp.tile([128, KH], FP32, tag="b1", name="b1")
            nc.gpsimd.dma_start(b1[:], b1_d.ap()[:])
            b2 = cp.tile([128, KH], FP32, tag="b2", name="b2")
            nc.gpsimd.dma_start(b2[:], b2_d.ap()[:])
            ck = cp.tile([128, KH], FP32, tag="ck", name="ck")
            nc.gpsimd.dma_start(ck[:], ck_d.ap()[:])
            cd = cp.tile([128, KH], FP32, tag="cd", name="cd")
            nc.gpsimd.dma_start(cd[:], cd_d.ap()[:])

            # ---- batch-resident inputs
            zTb = [sp.tile([128, BL], BF16, tag=f"zTb{k}", name=f"zTb{k}") for k in range(KD)]
            for c in range(NCHUNK):
                for k in range(KD):
                    nc.sync.dma_start(
                        zTb[k][:, c * CH : (c + 1) * CH],
                        zTb_d.ap()[k * 128 : (k + 1) * 128, c * CH : (c + 1) * CH],
                    )

            # ---- persistent per-chunk state
            T = [
                [sp.tile([128, CH], FP32, tag=f"T{c}_{m}", name=f"T{c}_{m}") for m in range(KH)]
                for c in range(NCHUNK)
            ]
            s = [
                [sp.tile([128, CH], BF16, tag=f"s{c}_{m}", name=f"s{c}_{m}") for m in range(KH)]
                for c in range(NCHUNK)
            ]

            def csl(c):
                return slice(c * CH, (c + 1) * CH)

            # ---- init: T = z @ W1q   (a_p = 0 since p0 = 0)
            for c in range(NCHUNK):
                for m in range(KH):
                    ps = pp.tile([128, CH], FP32, tag="mm", name="mm")
                    for k in range(KD):
                        nc.tensor.matmul(
                            ps[:],
                            w1q[:, k, m * 128 : (m + 1) * 128],
                            zTb[k][:, csl(c)],
                            start=(k == 0),
                            stop=(k == KD - 1),
                        )
                    nc.scalar.activation(T[c][m][:], ps[:], AF.Copy)

            # fp32 z arrives during the eval chain; only needed at the end
            zT = [sp.tile([128, BL], FP32, tag=f"zT{k}", name=f"zT{k}") for k in range(KD)]
            for k in range(KD):
                nc.gpsimd.dma_start(zT[k][:], zT_d.ap()[k * 128 : (k + 1) * 128, :])

            # ---- 8 gradient evals
            for ei, kind in enumerate(EVALS):
                # v_s = (-scale)*(h2^2 @ W2wneg) + (-scale)*C, u1s = (h1^2-1)*v_s
                neg_scale = (DT / 2) if kind == "k" else (-DT)
                cbias = ck if kind == "k" else cd
                updw = mqp if kind == "k" else mpq
                ndrift = sum(1 for x in EVALS[: ei + 1] if x == "d")
                is_last = ei == len(EVALS) - 1

                h1 = [
                    [wp.tile([128, CH], BF16, tag=f"h1_{c}_{m}", name=f"h1_{c}_{m}") for m in range(KH)]
                    for c in range(NCHUNK)
                ]
                sq1 = [
                    [wp.tile([128, CH], BF16, tag=f"sq1_{c}_{m}", name=f"sq1_{c}_{m}") for m in range(KH)]
                    for c in range(NCHUNK)
                ]
                h2 = [
                    [wp.tile([128, CH], BF16, tag=f"h2_{c}_{m}", name=f"h2_{c}_{m}") for m in range(KH)]
                    for c in range(NCHUNK)
                ]
                sq2 = [
                    [wp.tile([128, CH], BF16, tag=f"sq2_{c}_{m}", name=f"sq2_{c}_{m}") for m in range(KH)]
                    for c in range(NCHUNK)
                ]
                vs = [
                    [wp.tile([128, CH], BF16, tag=f"vs_{c}_{m}", name=f"vs_{c}_{m}") for m in range(KH)]
                    for c in range(NCHUNK)
                ]
                u1 = [
                    [wp.tile([128, CH], BF16, tag=f"u1_{c}_{m}", name=f"u1_{c}_{m}") for m in range(KH)]
                    for c in range(NCHUNK)
                ]

                for c in range(NCHUNK):
                    for m in range(KH):
                        nc.scalar.activation(
                            h1[c][m][:], T[c][m][:], AF.Tanh, bias=b1[:, m : m + 1]
                        )
                    for m in range(KH):
                        nc.vector.tensor_mul(sq1[c][m][:], h1[c][m][:], h1[c][m][:])

                for c in range(NCHUNK):
                    for m in range(KH):
                        ps = pp.tile([128, CH], FP32, tag="mm", name="mm")
                        for k in range(KH):
                            nc.tensor.matmul(
                                ps[:],
                                w2[:, k, m * 128 : (m + 1) * 128],
                                h1[c][k][:],
                                start=(k == 0),
                                stop=(k == KH - 1),
                            )
                        nc.scalar.activation(
                            h2[c][m][:], ps[:], AF.Tanh, bias=b2[:, m : m + 1]
                        )
                    for m in range(KH):
                        nc.vector.tensor_mul(sq2[c][m][:], h2[c][m][:], h2[c][m][:])

                for c in range(NCHUNK):
                    for m in range(KH):
                        ps = pp.tile([128, CH], FP32, tag="mm", name="mm")
                        for k in range(KH):
                            nc.tensor.matmul(
                                ps[:],
                                w2wn[:, k, m * 128 : (m + 1) * 128],
                                sq2[c][k][:],
                                start=(k == 0),
                                stop=(k == KH - 1),
                            )
                        nc.scalar.activation(
                            vs[c][m][:],
                            ps[:],
                            AF.Identity,
                            bias=cbias[:, m : m + 1],
                            scale=float(neg_scale),
                        )
                    for m in range(KH):
                        nc.vector.scalar_tensor_tensor(
                            u1[c][m][:],
                            sq1[c][m][:],
                            1.0,
                            vs[c][m][:],
                            ALU.subtract,
                            ALU.mult,
                        )

                # s accumulation on drift evals
                if kind == "d":
                    for c in range(NCHUNK):
                        for m in range(KH):
                            if ndrift == 1:
                                nc.vector.tensor_copy(s[c][m][:], u1[c][m][:])
                            else:
                                nc.vector.tensor_add(
                                    s[c][m][:], s[c][m][:], u1[c][m][:]
                                )

                # state update T += u1 @ updw (dead after the last drift),
                # else the final for this chunk: q = z + s @ W1p^T
                if not is_last:
                    for c in range(NCHUNK):
                        for m in range(KH):
                            ps = pp.tile([128, CH], FP32, tag="mm", name="mm")
                            for k in range(KH):
                                nc.tensor.matmul(
                                    ps[:],
                                    updw[:, k, m * 128 : (m + 1) * 128],
                                    u1[c][k][:],
                                    start=(k == 0),
                                    stop=(k == KH - 1),
                                )
                            nc.vector.tensor_add(T[c][m][:], T[c][m][:], ps[:])
                else:
                    for c in range(NCHUNK):
                        for mq in range(MQ):
                            ps = pf.tile([128, CH], FP32, tag="fin", name="fin")
                            for k in range(KH):
                                nc.tensor.matmul(
                                    ps[:],
                                    w1pt[:, k, mq * 128 : (mq + 1) * 128],
                                    s[c][k][:],
                                    start=(k == 0),
                                    stop=(k == KH - 1),
                                )
                            qo = qp.tile([128, CH], FP32, tag="qo", name="qo")
                            nc.vector.tensor_add(qo[:], zT[mq][:, csl(c)], ps[:])
                            nc.sync.dma_start(
                                qT_d.ap()[mq * 128 : (mq + 1) * 128, csl(c)], qo[:]
                            )

    nc.compile()
    return nc


_CACHE = {}


def _get_nc():
    if "nc" not in _CACHE:
        _CACHE["nc"] = build_nc()
    return _CACHE["nc"]


def _tile_k(a, ktiles):
    """[K, M] -> [128, ktiles, M] with K = ktiles*128 on partitions."""
    k, m = a.shape
    assert k == ktiles * 128
    return np.ascontiguousarray(a.reshape(ktiles, 128, m).transpose(1, 0, 2))


def _bias_tiles(v):
    """[256] -> [128, 2]: column m holds features m*128..(m+1)*128."""
    return np.ascontiguousarray(v.reshape(KH, 128).T)


def _prep_shared(W1, b1, W2, b2, W3, b3):
    W1 = np.asarray(W1, dtype=np.float32)
    W2 = np.asarray(W2, dtype=np.float32)
    w3 = np.asarray(W3, dtype=np.float32)[:, 0]
    b1 = np.asarray(b1, dtype=np.float32)
    b2 = np.asarray(b2, dtype=np.float32)
    W1q, W1p = W1[:DIM], W1[DIM:]
    W2wneg = -(w3[:, None] * W2.T)
    C = W2 @ w3
    Mqp = W1q.T @ W1p
    Mpq = W1p.T @ W1q
    return {
        "w1q": _tile_k(W1q, KD).astype(BF),
        "w2": _tile_k(W2, KH).astype(BF),
        "w2wn": _tile_k(W2wneg, KH).astype(BF),
        "mqp": _tile_k(Mqp, KH).astype(BF),
        "mpq": _tile_k(Mpq, KH).astype(BF),
        "w1pt": _tile_k(np.ascontiguousarray(W1p.T), KH).astype(BF),
        "b1": _bias_tiles(b1),
        "b2": _bias_tiles(b2),
        "ck": _bias_tiles((DT / 2) * C),
        "cd": _bias_tiles((-DT) * C),
    }


def run_kernel(z, W1, b1, W2, b2, W3, b3, trace=False, trace_cores=None):
    nc = _get_nc()
    shared = _prep_shared(W1, b1, W2, b2, W3, b3)
    z = np.asarray(z, dtype=np.float32)
    in_maps = []
    for i in range(N_CORES):
        zt = np.ascontiguousarray(z[i * BL : (i + 1) * BL].T)
        in_maps.append({**shared, "zT": zt, "zTb": zt.astype(BF)})
    res = run_bass_kernel_spmd(
        nc,
        in_maps,
        core_ids=list(range(N_CORES)),
        trace=trace,
        trace_cores=trace_cores,
    )
    out = np.concatenate(
        [res.results[i]["qT"].T for i in range(N_CORES)], axis=0
    )
    return np.ascontiguousarray(out), res


def kernel(z, W1, b1, W2, b2, W3, b3):
    out, _ = run_kernel(z, W1, b1, W2, b2, W3, b3)
    return out
